# revision 39
# baseline (speedup 1.0000x reference)
"""Trainium2 Bass kernel for nn_Decoupled_Block (dense transformer block).

Sharding (8 cores): tensor-parallel heads in the front, sequence-parallel tail.
  - Phase 1+2 (projections + attention): core g computes q/k/v and per-side
    q/k projections for head-dim slice g*128:(g+1)*128 (2 heads) of BOTH
    batches, runs the 3-branch max-fused attention for its 2 heads x 2
    batches over the full sequence, producing O^T slices [128, 1024] per
    batch.
  - One 8-rank AllToAll re-shards from head-slices to sequence-row slices:
    afterwards core j holds O^T[all 1024 c-dims, 256 q rows] for batch
    j//4, rows (j%4)*256:(j%4+1)*256.
  - Phase 3 (fc_o + LN1 + FF + LN2): each core computes the full tail for
    its 256 rows with full Wo/W1/W2. Output rows are gathered on the host.

Layout tricks:
  - All activations that feed matmul contractions are produced directly in
    "transposed" (contraction-on-partitions) layout; id_x / side are
    transposed on the host.
  - Biases are folded in by augmenting the contraction dim with a ones row
    (host-side), so projections need no separate bias pass.
  - Softmax over k with k on partitions: exp on ACT (no max subtraction
    needed; |scores| < ~1), Z row-sums via PE matmuls against a selection
    matrix, per-branch normalization deferred:
       A = max_n(e_n/Z_n) = r0 * max(e0, u1*e1, u2*e2),
    u_n = Z0/Zn and r0 = 1/Z0 computed via ln/exp (no reciprocal), r0
    applied after the A@V matmul on the [64, 1024] head output.
"""

import sys

for _p in ("/opt/trn_rl_repo",):
    if _p not in sys.path:
        sys.path.insert(0, _p)

from contextlib import ExitStack

import numpy as np
import ml_dtypes

import concourse.bacc as bacc
import concourse.bass as bass
import concourse.mybir as mybir
import concourse.tile as tile
from concourse.bass_utils import run_bass_kernel_spmd
from concourse.masks import make_identity

FP = mybir.dt.float32
BF = mybir.dt.bfloat16
AF = mybir.ActivationFunctionType
ALU = mybir.AluOpType

B, S, D, NS, H = 2, 1024, 1024, 2, 16
DH = D // H  # 64
NC = 8
CH = D // NC  # 128 head-dims (2 heads) per core
GSZ = 4
RQ = S // GSZ  # 256 output rows per core
DA = 1152  # augmented contraction dim (1024 + bias row + zero pad), 9 * 128
KT = 8  # 128-tiles of S / D
KTA = 9  # 128-tiles of DA
SCALE = 1.0 / np.sqrt(D)  # score scale (reference uses sqrt(dim), not head_dim)
LN_EPS = 1e-5
GELU_FUNC = None  # set in _build_nc; overridable for CoreSim (no Gelu there)
BF_NP = ml_dtypes.bfloat16
BUILD_PHASES = "all"  # "proj" | "attn" | "a2a" | "all" — debugging aid
SKIP_CC = False  # replace the AllToAll with a local DMA copy (debugging aid)

_CACHE = {}


def _build_nc():
    nc = bacc.Bacc(
        "TRN2",
        target_bir_lowering=False,
        debug=False,
        enable_asserts=False,
        num_devices=NC,
    )

    # ---- I/O ----
    xta = [
        nc.dram_tensor(f"xta{b}", [DA, S], BF, kind="ExternalInput") for b in range(B)
    ]
    sta = [
        [
            nc.dram_tensor(f"sta{b}_{n}", [DA, S], BF, kind="ExternalInput")
            for n in range(NS)
        ]
        for b in range(B)
    ]
    wqa = nc.dram_tensor("wqa", [DA, CH], BF, kind="ExternalInput")
    wka = nc.dram_tensor("wka", [DA, CH], BF, kind="ExternalInput")
    wva = nc.dram_tensor("wva", [DA, CH], BF, kind="ExternalInput")
    sqa = [
        nc.dram_tensor(f"sq{n}a", [DA, CH], BF, kind="ExternalInput")
        for n in range(NS)
    ]
    ska = [
        nc.dram_tensor(f"sk{n}a", [DA, CH], BF, kind="ExternalInput")
        for n in range(NS)
    ]
    wo = nc.dram_tensor("wo", [D, D], BF, kind="ExternalInput")
    w1a = nc.dram_tensor("w1a", [DA, D], BF, kind="ExternalInput")
    w2a = nc.dram_tensor("w2a", [DA, D], BF, kind="ExternalInput")
    idr = nc.dram_tensor("idr", [RQ, D], FP, kind="ExternalInput")
    g1r = nc.dram_tensor("g1r", [1, D], BF, kind="ExternalInput")
    be1r = nc.dram_tensor("be1r", [1, D], BF, kind="ExternalInput")
    g2r = nc.dram_tensor("g2r", [1, D], BF, kind="ExternalInput")
    be2r = nc.dram_tensor("be2r", [1, D], BF, kind="ExternalInput")
    cmatd = nc.dram_tensor("cmatd", [3, 96], FP, kind="ExternalInput")
    out = nc.dram_tensor("out", [RQ, D], FP, kind="ExternalOutput")

    def t9(ap):  # [DA, N] -> [128, 9, N]
        return ap.rearrange("(kt p) n -> p kt n", p=128)

    def t8(ap):  # [D, N] -> [128, 8, N]
        return ap.rearrange("(kt p) n -> p kt n", p=128)

    with tile.TileContext(nc) as tc, ExitStack() as top:
        # ---------- persistent pools ----------
        const = top.enter_context(tc.tile_pool(name="const", bufs=1))
        glob = top.enter_context(tc.tile_pool(name="glob", bufs=1))

        ident = const.tile([128, 128], FP)
        make_identity(nc, ident)
        ones128 = const.tile([128, 128], BF)
        nc.gpsimd.memset(ones128[:], 1.0)
        ones_row = ones128[0:1, :]
        # selection matrices for Z row-sum packing: sel[:, n, m] = (m == n)
        sel = const.tile([128, 3, 3], BF)
        nc.gpsimd.memset(sel[:], 0.0)
        for n in range(3):
            nc.gpsimd.memset(sel[:, n, n : n + 1], 1.0)
        # C matrix (lhsT) for u1@row0, u2@row32, r0@row64 = exp(C.T @ lnZ)
        epsc = const.tile([128, 1], FP)
        nc.gpsimd.memset(epsc[:], LN_EPS)
        cmat = const.tile([3, 96], FP)
        nc.sync.dma_start(cmat[:], cmatd.ap())

        # tiles that span the collective boundary
        otl = [glob.tile([128, S], BF, name=f"otl{b}") for b in range(B)]
        otg = glob.tile([128, KT, RQ], BF)
        xln = glob.tile([128, 2, D], FP)

        # ---------- phase 1+2: projections + attention ----------
        with ExitStack() as p12:
            qkv = p12.enter_context(tc.tile_pool(name="qkv", bufs=1))
            qT = [qkv.tile([128, S], BF, name=f"qT{b}") for b in range(B)]
            kT = [qkv.tile([128, S], BF, name=f"kT{b}") for b in range(B)]
            sqT = [
                [qkv.tile([128, S], BF, name=f"sqT{b}_{n}") for n in range(NS)]
                for b in range(B)
            ]
            skT = [
                [qkv.tile([128, S], BF, name=f"skT{b}_{n}") for n in range(NS)]
                for b in range(B)
            ]
            vnat = [qkv.tile([128, KT, CH], BF, name=f"vnat{b}") for b in range(B)]

            with ExitStack() as p1:
                acts = p1.enter_context(tc.tile_pool(name="acts", bufs=2))
                wpool = p1.enter_context(tc.tile_pool(name="wpool", bufs=1))
                pps = p1.enter_context(tc.tile_pool(name="pps", bufs=2, space="PSUM"))
                vps = p1.enter_context(tc.tile_pool(name="vps", bufs=2, space="PSUM"))

                # weight tiles stay resident for both batches
                w_sb = {}
                for name, dram in (
                    ("q", wqa),
                    ("k", wka),
                    ("v", wva),
                    ("sq0", sqa[0]),
                    ("sk0", ska[0]),
                    ("sq1", sqa[1]),
                    ("sk1", ska[1]),
                ):
                    w = wpool.tile([128, KTA, CH], BF, name=f"w_{name}")
                    nc.sync.dma_start(w[:], t9(dram.ap()))
                    w_sb[name] = w

                def projT(src_sb, w, outT, on_act):
                    """outT[128, S] = W_aug.T @ src_aug."""
                    ps = pps.tile([128, S], FP, tag="pps", name="projps")
                    for kt in range(KTA):
                        for nn in range(2):
                            nc.tensor.matmul(
                                ps[:, nn * 512 : (nn + 1) * 512],
                                lhsT=w[:, kt, :],
                                rhs=src_sb[:, kt, nn * 512 : (nn + 1) * 512],
                                start=(kt == 0),
                                stop=(kt == KTA - 1),
                            )
                    if on_act:
                        nc.scalar.copy(outT[:], ps[:])
                    else:
                        nc.vector.tensor_copy(outT[:], ps[:])

                for b in range(B):
                    xta_sb = acts.tile([128, KTA, S], BF, tag="act", name="xta_sb")
                    nc.sync.dma_start(xta_sb[:], t9(xta[b].ap()))
                    projT(xta_sb, w_sb["q"], qT[b], True)
                    projT(xta_sb, w_sb["k"], kT[b], False)
                    for st in range(KT):
                        ps = vps.tile([128, CH], FP, tag="vps", name="vprojps")
                        for kt in range(KTA):
                            nc.tensor.matmul(
                                ps[:],
                                lhsT=xta_sb[:, kt, st * 128 : (st + 1) * 128],
                                rhs=w_sb["v"][:, kt, :],
                                start=(kt == 0),
                                stop=(kt == KTA - 1),
                            )
                        nc.vector.tensor_copy(vnat[b][:, st, :], ps[:])
                    for n in range(NS):
                        sta_sb = acts.tile([128, KTA, S], BF, tag="act", name="sta_sb")
                        nc.sync.dma_start(sta_sb[:], t9(sta[b][n].ap()))
                        projT(sta_sb, w_sb[f"sq{n}"], sqT[b][n], True)
                        projT(sta_sb, w_sb[f"sk{n}"], skT[b][n], False)

            # ----- attention: 2 batches x 2 heads -----
            with ExitStack() as p2:
                epool = p2.enter_context(tc.tile_pool(name="epool", bufs=2))
                spool = p2.enter_context(
                    tc.tile_pool(name="spool", bufs=2, space="PSUM")
                )
                mpool = p2.enter_context(
                    tc.tile_pool(name="mpool", bufs=2, space="PSUM")
                )
                small = p2.enter_context(tc.tile_pool(name="small", bufs=2))

                for b in range(B):
                    for h in range(2):
                        off = h * DH
                        qhs = [qT[b], sqT[b][0], sqT[b][1]]
                        khs = [kT[b], skT[b][0], skT[b][1]]
                        e = epool.tile([128, 3, KT, S], BF, tag="e", name="e")
                        zpk = mpool.tile([3, S], FP, tag="m", name="zpk")
                        for n in range(3):
                            qh = qhs[n][off : off + DH, :]  # [64, S]
                            kh = khs[n][off : off + DH, :]
                            for kt in range(KT):
                                sps = spool.tile([128, S], FP, tag="s", name="sps")
                                for nn in range(2):
                                    nc.tensor.matmul(
                                        sps[:, nn * 512 : (nn + 1) * 512],
                                        lhsT=kh[:, kt * 128 : (kt + 1) * 128],
                                        rhs=qh[:, nn * 512 : (nn + 1) * 512],
                                        start=True,
                                        stop=True,
                                    )
                                nc.scalar.activation(
                                    e[:, n, kt, :], sps[:], AF.Exp, scale=SCALE
                                )
                                for nn in range(2):
                                    nc.tensor.matmul(
                                        zpk[:, nn * 512 : (nn + 1) * 512],
                                        lhsT=sel[:, n, :],
                                        rhs=e[:, n, kt, nn * 512 : (nn + 1) * 512],
                                        start=(n == 0 and kt == 0),
                                        stop=(n == 2 and kt == KT - 1),
                                    )
                        lnz = small.tile([3, S], FP, tag="lnz", name="lnz")
                        nc.scalar.activation(lnz[:], zpk[:], AF.Ln)
                        wps = mpool.tile([96, S], FP, tag="m", name="wps")
                        for nn in range(2):
                            nc.tensor.matmul(
                                wps[:, nn * 512 : (nn + 1) * 512],
                                lhsT=cmat[:],
                                rhs=lnz[:, nn * 512 : (nn + 1) * 512],
                                start=True,
                                stop=True,
                            )
                        u = small.tile([96, S], BF, tag="u", name="u")
                        nc.scalar.activation(u[:], wps[:], AF.Exp)
                        # broadcast u1@0, u2@32 (bf16), r0@64 (fp32)
                        ub = []
                        for j, (rows, dt_) in enumerate(
                            ((128, BF), (128, BF), (64, BF))
                        ):
                            p0 = 32 * j
                            bps = mpool.tile([128, S], FP, tag="m", name=f"bps{j}")
                            for nn in range(2):
                                nc.tensor.matmul(
                                    bps[:rows, nn * 512 : (nn + 1) * 512],
                                    lhsT=ones128[p0 : p0 + 1, 0:rows],
                                    rhs=u[p0 : p0 + 1, nn * 512 : (nn + 1) * 512],
                                    start=True,
                                    stop=True,
                                )
                            ubj = small.tile([128, S], dt_, tag=f"ub{j}", name=f"ub{j}")
                            if j < 2:
                                nc.vector.tensor_copy(ubj[:], bps[:])
                            else:
                                nc.scalar.copy(ubj[:rows, :], bps[:rows, :])
                            ub.append(ubj)
                        # fuse: e0 = max(e0, u1*e1, u2*e2)
                        for kt in range(KT):
                            nc.vector.tensor_tensor(
                                e[:, 1, kt, :], e[:, 1, kt, :], ub[0][:], ALU.mult
                            )
                            nc.vector.tensor_tensor(
                                e[:, 2, kt, :], e[:, 2, kt, :], ub[1][:], ALU.mult
                            )
                            nc.vector.tensor_tensor(
                                e[:, 0, kt, :], e[:, 0, kt, :], e[:, 1, kt, :], ALU.max
                            )
                            nc.vector.tensor_tensor(
                                e[:, 0, kt, :], e[:, 0, kt, :], e[:, 2, kt, :], ALU.max
                            )
                        # O^T_h [64, S] accumulation over k tiles
                        ops = mpool.tile([64, S], FP, tag="m", name="ops")
                        for kt in range(KT):
                            for nn in range(2):
                                nc.tensor.matmul(
                                    ops[:, nn * 512 : (nn + 1) * 512],
                                    lhsT=vnat[b][:, kt, off : off + DH],
                                    rhs=e[:, 0, kt, nn * 512 : (nn + 1) * 512],
                                    start=(kt == 0),
                                    stop=(kt == KT - 1),
                                )
                        nc.vector.tensor_tensor(
                            otl[b][off : off + DH, :], ops[:], ub[2][0:DH, :], ALU.mult
                        )

        if BUILD_PHASES in ("proj", "attn"):
            out_t = out.ap().rearrange("(m p) d -> p m d", p=128)
            if BUILD_PHASES == "proj":
                for b in range(B):
                    nc.sync.dma_start(out_t[:, b, :], otl[b][:])  # garbage; just runs
            else:
                for b in range(B):
                    nc.sync.dma_start(out_t[:, b, :], otl[b][:])
            nc.compile()
            return nc

        # ---------- all-to-all: head-slices -> row-slices ----------
        with ExitStack() as p3:
            dpool = p3.enter_context(tc.tile_pool(name="dpool", bufs=1, space="DRAM"))
            ccin = dpool.tile([NC, CH, RQ], BF, name="ccin")
            ccout = dpool.tile([NC, CH, RQ], BF, name="ccout")
            for b in range(B):
                nc.sync.dma_start(
                    ccin[b * GSZ : (b + 1) * GSZ].rearrange("j p w -> p j w"),
                    otl[b].rearrange("p (j w) -> p j w", j=GSZ),
                )
            if SKIP_CC:
                nc.gpsimd.dma_start(ccout[:], ccin[:])
            else:
                nc.gpsimd.collective_compute(
                    "AllToAll",
                    ALU.bypass,
                    replica_groups=[list(range(NC))],
                    ins=[ccin.opt()],
                    outs=[ccout.opt()],
                )
            nc.sync.dma_start(otg[:], ccout.rearrange("i p w -> p i w"))

        if BUILD_PHASES == "a2a":
            out_t = out.ap().rearrange("(m p) d -> p m d", p=128)
            for m in range(2):
                nc.sync.dma_start(out_t[:, m, :], otg[:, 2*m:2*m+2, :].rearrange("p a w -> p (a w)"))
            nc.compile()
            return nc

        # ---------- phase 4: fc_o + residual + LN1 ----------
        def layer_norm(x_ps, res_sb, gb, bb, out_sb, pool):
            """out = LN(x_ps + res_sb) * gb + bb   (one 128-row tile)."""
            x = pool.tile([128, D], FP, tag="lnx", name="lnx")
            nc.vector.tensor_tensor(x[:], x_ps[:], res_sb[:], ALU.add)
            sm = pool.tile([128, 1], FP, tag="ln_sm", name="sm")
            nc.vector.reduce_sum(sm[:], x[:], axis=mybir.AxisListType.X)
            nm = pool.tile([128, 1], FP, tag="ln_nm", name="nm")
            nc.vector.tensor_scalar_mul(nm[:], sm[:], -1.0 / D)
            junk = pool.tile([128, D], FP, tag="lnj", name="junk")
            ssq = pool.tile([128, 1], FP, tag="ln_ssq", name="ssq")
            nc.vector.tensor_tensor(junk[:], x[:], x[:], ALU.mult)
            nc.vector.reduce_sum(ssq[:], junk[:], axis=mybir.AxisListType.X)
            msq = pool.tile([128, 1], FP, tag="ln_msq", name="msq")
            nc.vector.tensor_tensor(msq[:], nm[:], nm[:], ALU.mult)
            var = pool.tile([128, 1], FP, tag="ln_var", name="var")
            # var = ssq/D - mean^2  (nm = -mean, msq = mean^2)
            nc.vector.scalar_tensor_tensor(
                out=var[:],
                in0=ssq[:],
                scalar=1.0 / D,
                in1=msq[:],
                op0=ALU.mult,
                op1=ALU.subtract,
            )
            lnv = pool.tile([128, 1], FP, tag="ln_lnv", name="lnv")
            nc.scalar.activation(lnv[:], var[:], AF.Ln, bias=epsc[:])
            rstd = pool.tile([128, 1], FP, tag="ln_rstd", name="rstd")
            nc.scalar.activation(rstd[:], lnv[:], AF.Exp, scale=-0.5)
            nmr = pool.tile([128, 1], FP, tag="ln_nmr", name="nmr")
            nc.vector.tensor_tensor(nmr[:], nm[:], rstd[:], ALU.mult)
            z = pool.tile([128, D], FP, tag="lnz2", name="z")
            nc.scalar.activation(z[:], x[:], AF.Identity, bias=nmr[:], scale=rstd[:])
            nc.vector.tensor_tensor(z[:], z[:], gb[:], ALU.mult)
            nc.vector.tensor_tensor(out_sb[:], z[:], bb[:], ALU.add)

        with ExitStack() as p4:
            wop = p4.enter_context(tc.tile_pool(name="wop", bufs=1))
            fps = p4.enter_context(tc.tile_pool(name="fps", bufs=2, space="PSUM"))
            lnp = p4.enter_context(tc.tile_pool(name="lnp", bufs=2))

            # affine broadcast tiles
            gb_tiles = []
            for nm_, dram in (("g1", g1r), ("be1", be1r), ("g2", g2r), ("be2", be2r)):
                row = lnp.tile([1, D], BF, tag="grow", name=f"{nm_}row")
                nc.sync.dma_start(row[:], dram.ap())
                bps = fps.tile([128, D], FP, tag="f", name=f"{nm_}bps")
                for nn in range(2):
                    nc.tensor.matmul(
                        bps[:, nn * 512 : (nn + 1) * 512],
                        lhsT=ones_row[:],
                        rhs=row[0:1, nn * 512 : (nn + 1) * 512],
                        start=True,
                        stop=True,
                    )
                gb = glob.tile([128, D], FP, name=f"{nm_}b")
                nc.scalar.copy(gb[:], bps[:])
                gb_tiles.append(gb)
            g1b, be1b, g2b, be2b = gb_tiles

            wo_sb = wop.tile([128, KT, D], BF)
            nc.sync.dma_start(wo_sb[:], t8(wo.ap()))
            idr_sb = glob.tile([128, 2, D], FP)
            nc.sync.dma_start(idr_sb[:], idr.ap().rearrange("(m p) d -> p m d", p=128))

            for m in range(2):
                ps = fps.tile([128, D], FP, tag="f", name="fcops")
                for kt in range(KT):
                    for nn in range(2):
                        nc.tensor.matmul(
                            ps[:, nn * 512 : (nn + 1) * 512],
                            lhsT=otg[:, kt, m * 128 : (m + 1) * 128],
                            rhs=wo_sb[:, kt, nn * 512 : (nn + 1) * 512],
                            start=(kt == 0),
                            stop=(kt == KT - 1),
                        )
                layer_norm(ps, idr_sb[:, m, :], g1b, be1b, xln[:, m, :], lnp)

        # ---------- phase 5: FF + residual + LN2 ----------
        with ExitStack() as p5:
            wfp = p5.enter_context(tc.tile_pool(name="wfp", bufs=1))
            hpool = p5.enter_context(tc.tile_pool(name="hpool", bufs=1))
            tps = p5.enter_context(tc.tile_pool(name="tps", bufs=2, space="PSUM"))
            f1ps = p5.enter_context(tc.tile_pool(name="f1ps", bufs=2, space="PSUM"))
            f2ps = p5.enter_context(tc.tile_pool(name="f2ps", bufs=2, space="PSUM"))
            lnp2 = p5.enter_context(tc.tile_pool(name="lnp2", bufs=2))

            # x_ln^T augmented [128, 9, RQ] (ones row at index 1024)
            xnt = hpool.tile([128, KTA, RQ], BF)
            nc.gpsimd.memset(xnt[:, KTA - 1, :], 0.0)
            nc.gpsimd.memset(xnt[0:1, KTA - 1, :], 1.0)
            for m in range(2):
                for dt_ in range(KT):
                    tp = tps.tile([128, 128], FP, tag="t", name="tp")
                    nc.tensor.transpose(
                        tp[:], xln[:, m, dt_ * 128 : (dt_ + 1) * 128], ident[:]
                    )
                    nc.vector.tensor_copy(xnt[:, dt_, m * 128 : (m + 1) * 128], tp[:])

            w1_sb = wfp.tile([128, KTA, D], BF, tag="w1")
            nc.sync.dma_start(w1_sb[:], t9(w1a.ap()))
            # h^T augmented [128, 9, RQ] with gelu applied
            hta = hpool.tile([128, KTA, RQ], BF)
            nc.gpsimd.memset(hta[:, KTA - 1, :], 0.0)
            nc.gpsimd.memset(hta[0:1, KTA - 1, :], 1.0)
            for mh in range(KT):
                ps = f1ps.tile([128, RQ], FP, tag="f1", name="ff1ps")
                for kt in range(KTA):
                    nc.tensor.matmul(
                        ps[:],
                        lhsT=w1_sb[:, kt, mh * 128 : (mh + 1) * 128],
                        rhs=xnt[:, kt, :],
                        start=(kt == 0),
                        stop=(kt == KTA - 1),
                    )
                nc.scalar.activation(hta[:, mh, :], ps[:], GELU_FUNC or AF.Gelu)

            w2_sb = wfp.tile([128, KTA, D], BF, tag="w2")
            nc.sync.dma_start(w2_sb[:], t9(w2a.ap()))
            out_t = out.ap().rearrange("(m p) d -> p m d", p=128)
            for m in range(2):
                ps = f2ps.tile([128, D], FP, tag="f2", name="ff2ps")
                for kt in range(KTA):
                    for nn in range(2):
                        nc.tensor.matmul(
                            ps[:, nn * 512 : (nn + 1) * 512],
                            lhsT=hta[:, kt, m * 128 : (m + 1) * 128],
                            rhs=w2_sb[:, kt, nn * 512 : (nn + 1) * 512],
                            start=(kt == 0),
                            stop=(kt == KTA - 1),
                        )
                osb = lnp2.tile([128, D], FP, tag="osb", name="osb")
                layer_norm(ps, xln[:, m, :], g2b, be2b, osb, lnp2)
                nc.sync.dma_start(out_t[:, m, :], osb[:])

    nc.compile()
    return nc


def _cmat_np():
    c = np.zeros((3, 96), np.float32)
    c[0, 0], c[1, 0] = 1.0, -1.0   # u1 = Z0/Z1 @ row 0
    c[0, 32], c[2, 32] = 1.0, -1.0  # u2 = Z0/Z2 @ row 32
    c[0, 64] = -1.0                 # r0 = 1/Z0 @ row 64
    return c


def _prep_inputs(id_x, side, Wq, bq, Wk, bk, Wv, bv, sWq, sbq, sWk, sbk, Wo, bo,
                 W1, b1, W2, b2, g1, be1, g2, be2):
    """Build the 8 per-core input maps."""
    f = np.float32
    id_x = np.asarray(id_x, f)
    side = np.asarray(side, f)

    def aug_act(xt):  # [D, S] -> [DA, S] bf16 with ones row at D
        a = np.zeros((DA, S), f)
        a[:D] = xt
        a[D] = 1.0
        return a.astype(BF_NP)

    def aug_w(W, b_, cs):  # [D, D], [D] -> [DA, CH] bf16
        a = np.zeros((DA, CH), f)
        a[:D] = np.asarray(W, f)[:, cs]
        a[D] = np.asarray(b_, f)[cs]
        return a.astype(BF_NP)

    def aug_w_full(W, b_):  # [D, D], [D] -> [DA, D] bf16
        a = np.zeros((DA, D), f)
        a[:D] = np.asarray(W, f)
        a[D] = np.asarray(b_, f)
        return a.astype(BF_NP)

    xta_b = [aug_act(np.ascontiguousarray(id_x[b].T)) for b in range(B)]
    sta_b = [
        [aug_act(np.ascontiguousarray(side[b, :, n, :].T)) for n in range(NS)]
        for b in range(B)
    ]
    w1a = aug_w_full(W1, b1)
    w2a = aug_w_full(W2, b2)
    wo_f = np.ascontiguousarray(np.asarray(Wo, f)).astype(BF_NP)
    bo_f = np.asarray(bo, f)

    in_maps = []
    for cid in range(NC):
        bq_, g = cid // GSZ, cid % GSZ
        cs = slice(cid * CH, (cid + 1) * CH)
        rows = slice(g * RQ, (g + 1) * RQ)
        m = {
            "xta0": xta_b[0],
            "xta1": xta_b[1],
            "sta0_0": sta_b[0][0],
            "sta0_1": sta_b[0][1],
            "sta1_0": sta_b[1][0],
            "sta1_1": sta_b[1][1],
            "wqa": aug_w(Wq, bq, cs),
            "wka": aug_w(Wk, bk, cs),
            "wva": aug_w(Wv, bv, cs),
            "sq0a": aug_w(np.asarray(sWq, f)[0], np.asarray(sbq, f)[0], cs),
            "sk0a": aug_w(np.asarray(sWk, f)[0], np.asarray(sbk, f)[0], cs),
            "sq1a": aug_w(np.asarray(sWq, f)[1], np.asarray(sbq, f)[1], cs),
            "sk1a": aug_w(np.asarray(sWk, f)[1], np.asarray(sbk, f)[1], cs),
            "wo": wo_f,
            "w1a": w1a,
            "w2a": w2a,
            "idr": np.ascontiguousarray(id_x[bq_, rows, :]) + bo_f[None, :].astype(f),
            "cmatd": _cmat_np(),
            "g1r": np.asarray(g1, f).reshape(1, D).astype(BF_NP),
            "be1r": np.asarray(be1, f).reshape(1, D).astype(BF_NP),
            "g2r": np.asarray(g2, f).reshape(1, D).astype(BF_NP),
            "be2r": np.asarray(be2, f).reshape(1, D).astype(BF_NP),
        }
        in_maps.append(m)
    return in_maps


def kernel(id_x, side, num_heads, Wq, bq, Wk, bk, Wv, bv, sWq, sbq, sWk, sbk,
           Wo, bo, W1, b1, W2, b2, g1, be1, g2, be2, _trace=False):
    assert int(num_heads) == H
    in_maps = _prep_inputs(id_x, side, Wq, bq, Wk, bk, Wv, bv, sWq, sbq, sWk, sbk,
                           Wo, bo, W1, b1, W2, b2, g1, be1, g2, be2)
    if "nc" not in _CACHE:
        _CACHE["nc"] = _build_nc()
    nc = _CACHE["nc"]
    res = run_bass_kernel_spmd(nc, in_maps, core_ids=list(range(NC)), trace=_trace)
    _CACHE["last_result"] = res
    outp = np.zeros((B, S, D), np.float32)
    for cid in range(NC):
        b, g = cid // GSZ, cid % GSZ
        outp[b, g * RQ : (g + 1) * RQ, :] = res.results[cid]["out"]
    return outp



# revision 40
# speedup vs baseline: 1.0066x; 1.0066x over previous
"""Trainium2 Bass kernel for nn_Decoupled_Block (dense transformer block).

Sharding (8 cores): tensor-parallel heads in the front, sequence-parallel tail.
  - Phase 1+2 (projections + attention): core g computes q/k/v and per-side
    q/k projections for head-dim slice g*128:(g+1)*128 (2 heads) of BOTH
    batches, runs the 3-branch max-fused attention for its 2 heads x 2
    batches over the full sequence, producing O^T slices [128, 1024] per
    batch.
  - One 8-rank AllToAll re-shards from head-slices to sequence-row slices:
    afterwards core j holds O^T[all 1024 c-dims, 256 q rows] for batch
    j//4, rows (j%4)*256:(j%4+1)*256.
  - Phase 3 (fc_o + LN1 + FF + LN2): each core computes the full tail for
    its 256 rows with full Wo/W1/W2. Output rows are gathered on the host.

Layout tricks:
  - All activations that feed matmul contractions are produced directly in
    "transposed" (contraction-on-partitions) layout; id_x / side are
    transposed on the host.
  - Biases are folded in by augmenting the contraction dim with a ones row
    (host-side), so projections need no separate bias pass.
  - Softmax over k with k on partitions: exp on ACT (no max subtraction
    needed; |scores| < ~1), Z row-sums via PE matmuls against a selection
    matrix, per-branch normalization deferred:
       A = max_n(e_n/Z_n) = r0 * max(e0, u1*e1, u2*e2),
    u_n = Z0/Zn and r0 = 1/Z0 computed via ln/exp (no reciprocal), r0
    applied after the A@V matmul on the [64, 1024] head output.
"""

import sys

for _p in ("/opt/trn_rl_repo",):
    if _p not in sys.path:
        sys.path.insert(0, _p)

from contextlib import ExitStack

import numpy as np
import ml_dtypes

import concourse.bacc as bacc
import concourse.bass as bass
import concourse.mybir as mybir
import concourse.tile as tile
from concourse.bass_utils import run_bass_kernel_spmd
from concourse.masks import make_identity

FP = mybir.dt.float32
BF = mybir.dt.bfloat16
AF = mybir.ActivationFunctionType
ALU = mybir.AluOpType

B, S, D, NS, H = 2, 1024, 1024, 2, 16
DH = D // H  # 64
NC = 8
CH = D // NC  # 128 head-dims (2 heads) per core
GSZ = 4
RQ = S // GSZ  # 256 output rows per core
DA = 1152  # augmented contraction dim (1024 + bias row + zero pad), 9 * 128
KT = 8  # 128-tiles of S / D
KTA = 9  # 128-tiles of DA
SCALE = 1.0 / np.sqrt(D)  # score scale (reference uses sqrt(dim), not head_dim)
LN_EPS = 1e-5
GELU_FUNC = None  # set in _build_nc; overridable for CoreSim (no Gelu there)
BF_NP = ml_dtypes.bfloat16
BUILD_PHASES = "all"  # "proj" | "attn" | "a2a" | "all" — debugging aid
SKIP_CC = False  # replace the AllToAll with a local DMA copy (debugging aid)

_CACHE = {}


def _build_nc():
    nc = bacc.Bacc(
        "TRN2",
        target_bir_lowering=False,
        debug=False,
        enable_asserts=False,
        num_devices=NC,
    )

    # ---- I/O ----
    xta = [
        nc.dram_tensor(f"xta{b}", [DA, S], BF, kind="ExternalInput") for b in range(B)
    ]
    sta = [
        [
            nc.dram_tensor(f"sta{b}_{n}", [DA, S], BF, kind="ExternalInput")
            for n in range(NS)
        ]
        for b in range(B)
    ]
    wqa = nc.dram_tensor("wqa", [DA, CH], BF, kind="ExternalInput")
    wka = nc.dram_tensor("wka", [DA, CH], BF, kind="ExternalInput")
    wva = nc.dram_tensor("wva", [DA, CH], BF, kind="ExternalInput")
    sqa = [
        nc.dram_tensor(f"sq{n}a", [DA, CH], BF, kind="ExternalInput")
        for n in range(NS)
    ]
    ska = [
        nc.dram_tensor(f"sk{n}a", [DA, CH], BF, kind="ExternalInput")
        for n in range(NS)
    ]
    wo = nc.dram_tensor("wo", [D, D], BF, kind="ExternalInput")
    w1a = nc.dram_tensor("w1a", [DA, D], BF, kind="ExternalInput")
    w2a = nc.dram_tensor("w2a", [DA, D], BF, kind="ExternalInput")
    idr = nc.dram_tensor("idr", [RQ, D], FP, kind="ExternalInput")
    g1r = nc.dram_tensor("g1r", [1, D], BF, kind="ExternalInput")
    be1r = nc.dram_tensor("be1r", [1, D], BF, kind="ExternalInput")
    g2r = nc.dram_tensor("g2r", [1, D], BF, kind="ExternalInput")
    be2r = nc.dram_tensor("be2r", [1, D], BF, kind="ExternalInput")
    cmatd = nc.dram_tensor("cmatd", [3, 96], FP, kind="ExternalInput")
    out = nc.dram_tensor("out", [RQ, D], FP, kind="ExternalOutput")

    def t9(ap):  # [DA, N] -> [128, 9, N]
        return ap.rearrange("(kt p) n -> p kt n", p=128)

    def t8(ap):  # [D, N] -> [128, 8, N]
        return ap.rearrange("(kt p) n -> p kt n", p=128)

    with tile.TileContext(nc) as tc, ExitStack() as top:
        # ---------- persistent pools ----------
        const = top.enter_context(tc.tile_pool(name="const", bufs=1))
        glob = top.enter_context(tc.tile_pool(name="glob", bufs=1))

        ident = const.tile([128, 128], FP)
        make_identity(nc, ident)
        ones128 = const.tile([128, 128], BF)
        nc.gpsimd.memset(ones128[:], 1.0)
        ones_row = ones128[0:1, :]
        # selection matrices for Z row-sum packing: sel[:, n, m] = (m == n)
        sel = const.tile([128, 3, 3], BF)
        nc.gpsimd.memset(sel[:], 0.0)
        for n in range(3):
            nc.gpsimd.memset(sel[:, n, n : n + 1], 1.0)
        # C matrix (lhsT) for u1@row0, u2@row32, r0@row64 = exp(C.T @ lnZ)
        epsc = const.tile([128, 1], FP)
        nc.gpsimd.memset(epsc[:], LN_EPS)
        cmat = const.tile([3, 96], FP)
        nc.sync.dma_start(cmat[:], cmatd.ap())

        # tiles that span the collective boundary
        otl = [glob.tile([128, S], BF, name=f"otl{b}") for b in range(B)]
        otg = glob.tile([128, KT, RQ], BF)
        xln = glob.tile([128, 2, D], FP)

        # ---------- phase 1+2: projections + attention ----------
        with ExitStack() as p12:
            qkv = p12.enter_context(tc.tile_pool(name="qkv", bufs=1))
            qT = [qkv.tile([128, S], BF, name=f"qT{b}") for b in range(B)]
            kT = [qkv.tile([128, S], BF, name=f"kT{b}") for b in range(B)]
            sqT = [
                [qkv.tile([128, S], BF, name=f"sqT{b}_{n}") for n in range(NS)]
                for b in range(B)
            ]
            skT = [
                [qkv.tile([128, S], BF, name=f"skT{b}_{n}") for n in range(NS)]
                for b in range(B)
            ]
            vnat = [qkv.tile([128, KT, CH], BF, name=f"vnat{b}") for b in range(B)]

            with ExitStack() as p1:
                acts = p1.enter_context(tc.tile_pool(name="acts", bufs=2))
                wpool = p1.enter_context(tc.tile_pool(name="wpool", bufs=1))
                pps = p1.enter_context(tc.tile_pool(name="pps", bufs=2, space="PSUM"))
                vps = p1.enter_context(tc.tile_pool(name="vps", bufs=2, space="PSUM"))

                # weight tiles stay resident for both batches
                w_sb = {}
                for name, dram in (
                    ("q", wqa),
                    ("k", wka),
                    ("v", wva),
                    ("sq0", sqa[0]),
                    ("sk0", ska[0]),
                    ("sq1", sqa[1]),
                    ("sk1", ska[1]),
                ):
                    w = wpool.tile([128, KTA, CH], BF, name=f"w_{name}")
                    nc.sync.dma_start(w[:], t9(dram.ap()))
                    w_sb[name] = w

                def projT(src_sb, w, outT, on_act):
                    """outT[128, S] = W_aug.T @ src_aug."""
                    ps = pps.tile([128, S], FP, tag="pps", name="projps")
                    for kt in range(KTA):
                        for nn in range(2):
                            nc.tensor.matmul(
                                ps[:, nn * 512 : (nn + 1) * 512],
                                lhsT=w[:, kt, :],
                                rhs=src_sb[:, kt, nn * 512 : (nn + 1) * 512],
                                start=(kt == 0),
                                stop=(kt == KTA - 1),
                            )
                    if on_act:
                        nc.scalar.copy(outT[:], ps[:])
                    else:
                        nc.vector.tensor_copy(outT[:], ps[:])

                for b in range(B):
                    xta_sb = acts.tile([128, KTA, S], BF, tag="act", name="xta_sb")
                    nc.sync.dma_start(xta_sb[:], t9(xta[b].ap()))
                    projT(xta_sb, w_sb["q"], qT[b], True)
                    projT(xta_sb, w_sb["k"], kT[b], False)
                    for st in range(KT):
                        ps = vps.tile([128, CH], FP, tag="vps", name="vprojps")
                        for kt in range(KTA):
                            nc.tensor.matmul(
                                ps[:],
                                lhsT=xta_sb[:, kt, st * 128 : (st + 1) * 128],
                                rhs=w_sb["v"][:, kt, :],
                                start=(kt == 0),
                                stop=(kt == KTA - 1),
                            )
                        nc.vector.tensor_copy(vnat[b][:, st, :], ps[:])
                    for n in range(NS):
                        sta_sb = acts.tile([128, KTA, S], BF, tag="act", name="sta_sb")
                        nc.sync.dma_start(sta_sb[:], t9(sta[b][n].ap()))
                        projT(sta_sb, w_sb[f"sq{n}"], sqT[b][n], True)
                        projT(sta_sb, w_sb[f"sk{n}"], skT[b][n], False)

            # ----- attention: 2 batches x 2 heads -----
            with ExitStack() as p2:
                epool = p2.enter_context(tc.tile_pool(name="epool", bufs=2))
                spool = p2.enter_context(
                    tc.tile_pool(name="spool", bufs=2, space="PSUM")
                )
                mpool = p2.enter_context(
                    tc.tile_pool(name="mpool", bufs=2, space="PSUM")
                )
                small = p2.enter_context(tc.tile_pool(name="small", bufs=2))

                for b in range(B):
                    for h in range(2):
                        off = h * DH
                        qhs = [qT[b], sqT[b][0], sqT[b][1]]
                        khs = [kT[b], skT[b][0], skT[b][1]]
                        e = epool.tile([128, 3, KT, S], BF, tag="e", name="e")
                        zpk = mpool.tile([3, S], FP, tag="m", name="zpk")
                        for n in range(3):
                            qh = qhs[n][off : off + DH, :]  # [64, S]
                            kh = khs[n][off : off + DH, :]
                            for kt in range(KT):
                                sps = spool.tile([128, S], FP, tag="s", name="sps")
                                for nn in range(2):
                                    nc.tensor.matmul(
                                        sps[:, nn * 512 : (nn + 1) * 512],
                                        lhsT=kh[:, kt * 128 : (kt + 1) * 128],
                                        rhs=qh[:, nn * 512 : (nn + 1) * 512],
                                        start=True,
                                        stop=True,
                                    )
                                nc.scalar.activation(
                                    e[:, n, kt, :], sps[:], AF.Exp, scale=SCALE
                                )
                                for nn in range(2):
                                    nc.tensor.matmul(
                                        zpk[:, nn * 512 : (nn + 1) * 512],
                                        lhsT=sel[:, n, :],
                                        rhs=e[:, n, kt, nn * 512 : (nn + 1) * 512],
                                        start=(n == 0 and kt == 0),
                                        stop=(n == 2 and kt == KT - 1),
                                    )
                        lnz = small.tile([3, S], FP, tag="lnz", name="lnz")
                        nc.scalar.activation(lnz[:], zpk[:], AF.Ln)
                        wps = mpool.tile([96, S], FP, tag="m", name="wps")
                        for nn in range(2):
                            nc.tensor.matmul(
                                wps[:, nn * 512 : (nn + 1) * 512],
                                lhsT=cmat[:],
                                rhs=lnz[:, nn * 512 : (nn + 1) * 512],
                                start=True,
                                stop=True,
                            )
                        u = small.tile([96, S], BF, tag="u", name="u")
                        nc.scalar.activation(u[:], wps[:], AF.Exp)
                        # broadcast u1@0, u2@32 (bf16), r0@64 (fp32)
                        ub = []
                        for j, (rows, dt_) in enumerate(
                            ((128, BF), (128, BF), (64, BF))
                        ):
                            p0 = 32 * j
                            bps = mpool.tile([128, S], FP, tag="m", name=f"bps{j}")
                            for nn in range(2):
                                nc.tensor.matmul(
                                    bps[:rows, nn * 512 : (nn + 1) * 512],
                                    lhsT=ones128[p0 : p0 + 1, 0:rows],
                                    rhs=u[p0 : p0 + 1, nn * 512 : (nn + 1) * 512],
                                    start=True,
                                    stop=True,
                                )
                            ubj = small.tile([128, S], dt_, tag=f"ub{j}", name=f"ub{j}")
                            if j < 2:
                                nc.vector.tensor_copy(ubj[:], bps[:])
                            else:
                                nc.scalar.copy(ubj[:rows, :], bps[:rows, :])
                            ub.append(ubj)
                        # fuse: e0 = max(e0, u1*e1, u2*e2)
                        for kt in range(KT):
                            nc.vector.tensor_tensor(
                                e[:, 1, kt, :], e[:, 1, kt, :], ub[0][:], ALU.mult
                            )
                            nc.vector.tensor_tensor(
                                e[:, 2, kt, :], e[:, 2, kt, :], ub[1][:], ALU.mult
                            )
                            nc.vector.tensor_tensor(
                                e[:, 0, kt, :], e[:, 0, kt, :], e[:, 1, kt, :], ALU.max
                            )
                            nc.vector.tensor_tensor(
                                e[:, 0, kt, :], e[:, 0, kt, :], e[:, 2, kt, :], ALU.max
                            )
                        # O^T_h [64, S] accumulation over k tiles
                        ops = mpool.tile([64, S], FP, tag="m", name="ops")
                        for kt in range(KT):
                            for nn in range(2):
                                nc.tensor.matmul(
                                    ops[:, nn * 512 : (nn + 1) * 512],
                                    lhsT=vnat[b][:, kt, off : off + DH],
                                    rhs=e[:, 0, kt, nn * 512 : (nn + 1) * 512],
                                    start=(kt == 0),
                                    stop=(kt == KT - 1),
                                )
                        nc.vector.tensor_tensor(
                            otl[b][off : off + DH, :], ops[:], ub[2][0:DH, :], ALU.mult
                        )

        if BUILD_PHASES in ("proj", "attn"):
            out_t = out.ap().rearrange("(m p) d -> p m d", p=128)
            if BUILD_PHASES == "proj":
                for b in range(B):
                    nc.sync.dma_start(out_t[:, b, :], otl[b][:])  # garbage; just runs
            else:
                for b in range(B):
                    nc.sync.dma_start(out_t[:, b, :], otl[b][:])
            nc.compile()
            return nc

        # ---------- all-to-all: head-slices -> row-slices ----------
        with ExitStack() as p3:
            dpool = p3.enter_context(tc.tile_pool(name="dpool", bufs=1, space="DRAM"))
            ccin = dpool.tile([NC, CH, RQ], BF, name="ccin")
            ccout = dpool.tile([NC, CH, RQ], BF, name="ccout")
            for b in range(B):
                nc.sync.dma_start(
                    ccin[b * GSZ : (b + 1) * GSZ].rearrange("j p w -> p j w"),
                    otl[b].rearrange("p (j w) -> p j w", j=GSZ),
                )
            if SKIP_CC:
                nc.gpsimd.dma_start(ccout[:], ccin[:])
            else:
                nc.gpsimd.collective_compute(
                    "AllToAll",
                    ALU.bypass,
                    replica_groups=[list(range(NC))],
                    ins=[ccin.opt()],
                    outs=[ccout.opt()],
                )
            nc.sync.dma_start(otg[:], ccout.rearrange("i p w -> p i w"))

        if BUILD_PHASES == "a2a":
            out_t = out.ap().rearrange("(m p) d -> p m d", p=128)
            for m in range(2):
                nc.sync.dma_start(out_t[:, m, :], otg[:, 2*m:2*m+2, :].rearrange("p a w -> p (a w)"))
            nc.compile()
            return nc

        # ---------- phase 4: fc_o + residual + LN1 ----------
        def layer_norm(x_ps, res_sb, gb, bb, out_sb, pool):
            """out = LN(x_ps + res_sb) * gb + bb   (one 128-row tile)."""
            x = pool.tile([128, D], FP, tag="lnx", name="lnx")
            nc.vector.tensor_tensor(x[:], x_ps[:], res_sb[:], ALU.add)
            sm = pool.tile([128, 1], FP, tag="ln_sm", name="sm")
            nc.vector.reduce_sum(sm[:], x[:], axis=mybir.AxisListType.X)
            nm = pool.tile([128, 1], FP, tag="ln_nm", name="nm")
            nc.vector.tensor_scalar_mul(nm[:], sm[:], -1.0 / D)
            junk = pool.tile([128, D], FP, tag="lnj", name="junk")
            ssq = pool.tile([128, 1], FP, tag="ln_ssq", name="ssq")
            nc.vector.tensor_tensor(junk[:], x[:], x[:], ALU.mult)
            nc.vector.reduce_sum(ssq[:], junk[:], axis=mybir.AxisListType.X)
            msq = pool.tile([128, 1], FP, tag="ln_msq", name="msq")
            nc.vector.tensor_tensor(msq[:], nm[:], nm[:], ALU.mult)
            var = pool.tile([128, 1], FP, tag="ln_var", name="var")
            # var = ssq/D - mean^2  (nm = -mean, msq = mean^2)
            nc.vector.scalar_tensor_tensor(
                out=var[:],
                in0=ssq[:],
                scalar=1.0 / D,
                in1=msq[:],
                op0=ALU.mult,
                op1=ALU.subtract,
            )
            sqv = pool.tile([128, 1], FP, tag="ln_lnv", name="sqv")
            nc.scalar.activation(sqv[:], var[:], AF.Sqrt, bias=epsc[:])
            rstd = pool.tile([128, 1], FP, tag="ln_rstd", name="rstd")
            nc.vector.reciprocal(rstd[:], sqv[:])
            nmr = pool.tile([128, 1], FP, tag="ln_nmr", name="nmr")
            nc.vector.tensor_tensor(nmr[:], nm[:], rstd[:], ALU.mult)
            z = pool.tile([128, D], FP, tag="lnz2", name="z")
            nc.vector.tensor_scalar(z[:], x[:], nmr[:], rstd[:], ALU.add, ALU.mult)
            nc.vector.tensor_tensor(z[:], z[:], gb[:], ALU.mult)
            nc.vector.tensor_tensor(out_sb[:], z[:], bb[:], ALU.add)

        with ExitStack() as p4:
            wop = p4.enter_context(tc.tile_pool(name="wop", bufs=1))
            fps = p4.enter_context(tc.tile_pool(name="fps", bufs=2, space="PSUM"))
            lnp = p4.enter_context(tc.tile_pool(name="lnp", bufs=2))

            # affine broadcast tiles
            gb_tiles = []
            for nm_, dram in (("g1", g1r), ("be1", be1r), ("g2", g2r), ("be2", be2r)):
                row = lnp.tile([1, D], BF, tag="grow", name=f"{nm_}row")
                nc.sync.dma_start(row[:], dram.ap())
                bps = fps.tile([128, D], FP, tag="f", name=f"{nm_}bps")
                for nn in range(2):
                    nc.tensor.matmul(
                        bps[:, nn * 512 : (nn + 1) * 512],
                        lhsT=ones_row[:],
                        rhs=row[0:1, nn * 512 : (nn + 1) * 512],
                        start=True,
                        stop=True,
                    )
                gb = glob.tile([128, D], FP, name=f"{nm_}b")
                nc.scalar.copy(gb[:], bps[:])
                gb_tiles.append(gb)
            g1b, be1b, g2b, be2b = gb_tiles

            wo_sb = wop.tile([128, KT, D], BF)
            nc.sync.dma_start(wo_sb[:], t8(wo.ap()))
            idr_sb = glob.tile([128, 2, D], FP)
            nc.sync.dma_start(idr_sb[:], idr.ap().rearrange("(m p) d -> p m d", p=128))

            for m in range(2):
                ps = fps.tile([128, D], FP, tag="f", name="fcops")
                for kt in range(KT):
                    for nn in range(2):
                        nc.tensor.matmul(
                            ps[:, nn * 512 : (nn + 1) * 512],
                            lhsT=otg[:, kt, m * 128 : (m + 1) * 128],
                            rhs=wo_sb[:, kt, nn * 512 : (nn + 1) * 512],
                            start=(kt == 0),
                            stop=(kt == KT - 1),
                        )
                layer_norm(ps, idr_sb[:, m, :], g1b, be1b, xln[:, m, :], lnp)

        # ---------- phase 5: FF + residual + LN2 ----------
        with ExitStack() as p5:
            wfp = p5.enter_context(tc.tile_pool(name="wfp", bufs=1))
            hpool = p5.enter_context(tc.tile_pool(name="hpool", bufs=1))
            tps = p5.enter_context(tc.tile_pool(name="tps", bufs=2, space="PSUM"))
            f1ps = p5.enter_context(tc.tile_pool(name="f1ps", bufs=2, space="PSUM"))
            f2ps = p5.enter_context(tc.tile_pool(name="f2ps", bufs=2, space="PSUM"))
            lnp2 = p5.enter_context(tc.tile_pool(name="lnp2", bufs=2))

            # x_ln^T augmented [128, 9, RQ] (ones row at index 1024)
            xnt = hpool.tile([128, KTA, RQ], BF)
            nc.gpsimd.memset(xnt[:, KTA - 1, :], 0.0)
            nc.gpsimd.memset(xnt[0:1, KTA - 1, :], 1.0)
            for m in range(2):
                for dt_ in range(KT):
                    tp = tps.tile([128, 128], FP, tag="t", name="tp")
                    nc.tensor.transpose(
                        tp[:], xln[:, m, dt_ * 128 : (dt_ + 1) * 128], ident[:]
                    )
                    nc.vector.tensor_copy(xnt[:, dt_, m * 128 : (m + 1) * 128], tp[:])

            w1_sb = wfp.tile([128, KTA, D], BF, tag="w1")
            nc.sync.dma_start(w1_sb[:], t9(w1a.ap()))
            # h^T augmented [128, 9, RQ] with gelu applied
            hta = hpool.tile([128, KTA, RQ], BF)
            nc.gpsimd.memset(hta[:, KTA - 1, :], 0.0)
            nc.gpsimd.memset(hta[0:1, KTA - 1, :], 1.0)
            for mh in range(KT):
                ps = f1ps.tile([128, RQ], FP, tag="f1", name="ff1ps")
                for kt in range(KTA):
                    nc.tensor.matmul(
                        ps[:],
                        lhsT=w1_sb[:, kt, mh * 128 : (mh + 1) * 128],
                        rhs=xnt[:, kt, :],
                        start=(kt == 0),
                        stop=(kt == KTA - 1),
                    )
                nc.scalar.activation(hta[:, mh, :], ps[:], GELU_FUNC or AF.Gelu)

            w2_sb = wfp.tile([128, KTA, D], BF, tag="w2")
            nc.sync.dma_start(w2_sb[:], t9(w2a.ap()))
            out_t = out.ap().rearrange("(m p) d -> p m d", p=128)
            for m in range(2):
                ps = f2ps.tile([128, D], FP, tag="f2", name="ff2ps")
                for kt in range(KTA):
                    for nn in range(2):
                        nc.tensor.matmul(
                            ps[:, nn * 512 : (nn + 1) * 512],
                            lhsT=hta[:, kt, m * 128 : (m + 1) * 128],
                            rhs=w2_sb[:, kt, nn * 512 : (nn + 1) * 512],
                            start=(kt == 0),
                            stop=(kt == KTA - 1),
                        )
                osb = lnp2.tile([128, D], FP, tag="osb", name="osb")
                layer_norm(ps, xln[:, m, :], g2b, be2b, osb, lnp2)
                nc.sync.dma_start(out_t[:, m, :], osb[:])

    nc.compile()
    return nc


def _cmat_np():
    c = np.zeros((3, 96), np.float32)
    c[0, 0], c[1, 0] = 1.0, -1.0   # u1 = Z0/Z1 @ row 0
    c[0, 32], c[2, 32] = 1.0, -1.0  # u2 = Z0/Z2 @ row 32
    c[0, 64] = -1.0                 # r0 = 1/Z0 @ row 64
    return c


def _prep_inputs(id_x, side, Wq, bq, Wk, bk, Wv, bv, sWq, sbq, sWk, sbk, Wo, bo,
                 W1, b1, W2, b2, g1, be1, g2, be2):
    """Build the 8 per-core input maps."""
    f = np.float32
    id_x = np.asarray(id_x, f)
    side = np.asarray(side, f)

    def aug_act(xt):  # [D, S] -> [DA, S] bf16 with ones row at D
        a = np.zeros((DA, S), f)
        a[:D] = xt
        a[D] = 1.0
        return a.astype(BF_NP)

    def aug_w(W, b_, cs):  # [D, D], [D] -> [DA, CH] bf16
        a = np.zeros((DA, CH), f)
        a[:D] = np.asarray(W, f)[:, cs]
        a[D] = np.asarray(b_, f)[cs]
        return a.astype(BF_NP)

    def aug_w_full(W, b_):  # [D, D], [D] -> [DA, D] bf16
        a = np.zeros((DA, D), f)
        a[:D] = np.asarray(W, f)
        a[D] = np.asarray(b_, f)
        return a.astype(BF_NP)

    xta_b = [aug_act(np.ascontiguousarray(id_x[b].T)) for b in range(B)]
    sta_b = [
        [aug_act(np.ascontiguousarray(side[b, :, n, :].T)) for n in range(NS)]
        for b in range(B)
    ]
    w1a = aug_w_full(W1, b1)
    w2a = aug_w_full(W2, b2)
    wo_f = np.ascontiguousarray(np.asarray(Wo, f)).astype(BF_NP)
    bo_f = np.asarray(bo, f)

    in_maps = []
    for cid in range(NC):
        bq_, g = cid // GSZ, cid % GSZ
        cs = slice(cid * CH, (cid + 1) * CH)
        rows = slice(g * RQ, (g + 1) * RQ)
        m = {
            "xta0": xta_b[0],
            "xta1": xta_b[1],
            "sta0_0": sta_b[0][0],
            "sta0_1": sta_b[0][1],
            "sta1_0": sta_b[1][0],
            "sta1_1": sta_b[1][1],
            "wqa": aug_w(Wq, bq, cs),
            "wka": aug_w(Wk, bk, cs),
            "wva": aug_w(Wv, bv, cs),
            "sq0a": aug_w(np.asarray(sWq, f)[0], np.asarray(sbq, f)[0], cs),
            "sk0a": aug_w(np.asarray(sWk, f)[0], np.asarray(sbk, f)[0], cs),
            "sq1a": aug_w(np.asarray(sWq, f)[1], np.asarray(sbq, f)[1], cs),
            "sk1a": aug_w(np.asarray(sWk, f)[1], np.asarray(sbk, f)[1], cs),
            "wo": wo_f,
            "w1a": w1a,
            "w2a": w2a,
            "idr": np.ascontiguousarray(id_x[bq_, rows, :]) + bo_f[None, :].astype(f),
            "cmatd": _cmat_np(),
            "g1r": np.asarray(g1, f).reshape(1, D).astype(BF_NP),
            "be1r": np.asarray(be1, f).reshape(1, D).astype(BF_NP),
            "g2r": np.asarray(g2, f).reshape(1, D).astype(BF_NP),
            "be2r": np.asarray(be2, f).reshape(1, D).astype(BF_NP),
        }
        in_maps.append(m)
    return in_maps


def kernel(id_x, side, num_heads, Wq, bq, Wk, bk, Wv, bv, sWq, sbq, sWk, sbk,
           Wo, bo, W1, b1, W2, b2, g1, be1, g2, be2, _trace=False):
    assert int(num_heads) == H
    in_maps = _prep_inputs(id_x, side, Wq, bq, Wk, bk, Wv, bv, sWq, sbq, sWk, sbk,
                           Wo, bo, W1, b1, W2, b2, g1, be1, g2, be2)
    if "nc" not in _CACHE:
        _CACHE["nc"] = _build_nc()
    nc = _CACHE["nc"]
    res = run_bass_kernel_spmd(nc, in_maps, core_ids=list(range(NC)), trace=_trace)
    _CACHE["last_result"] = res
    outp = np.zeros((B, S, D), np.float32)
    for cid in range(NC):
        b, g = cid // GSZ, cid % GSZ
        outp[b, g * RQ : (g + 1) * RQ, :] = res.results[cid]["out"]
    return outp



# revision 41
# speedup vs baseline: 1.0333x; 1.0265x over previous
"""Trainium2 Bass kernel for nn_Decoupled_Block (dense transformer block).

Sharding (8 cores): tensor-parallel heads in the front, sequence-parallel tail.
  - Phase 1+2 (projections + attention): core g computes q/k/v and per-side
    q/k projections for head-dim slice g*128:(g+1)*128 (2 heads) of BOTH
    batches, runs the 3-branch max-fused attention for its 2 heads x 2
    batches over the full sequence, producing O^T slices [128, 1024] per
    batch.
  - One 8-rank AllToAll re-shards from head-slices to sequence-row slices:
    afterwards core j holds O^T[all 1024 c-dims, 256 q rows] for batch
    j//4, rows (j%4)*256:(j%4+1)*256.
  - Phase 3 (fc_o + LN1 + FF + LN2): each core computes the full tail for
    its 256 rows with full Wo/W1/W2. Output rows are gathered on the host.

Layout tricks:
  - All activations that feed matmul contractions are produced directly in
    "transposed" (contraction-on-partitions) layout; id_x / side are
    transposed on the host.
  - Biases are folded in by augmenting the contraction dim with a ones row
    (host-side), so projections need no separate bias pass.
  - Softmax over k with k on partitions: exp on ACT (no max subtraction
    needed; |scores| < ~1), Z row-sums via PE matmuls against a selection
    matrix, per-branch normalization deferred:
       A = max_n(e_n/Z_n) = r0 * max(e0, u1*e1, u2*e2),
    u_n = Z0/Zn and r0 = 1/Z0 computed via ln/exp (no reciprocal), r0
    applied after the A@V matmul on the [64, 1024] head output.
"""

import sys

for _p in ("/opt/trn_rl_repo",):
    if _p not in sys.path:
        sys.path.insert(0, _p)

from contextlib import ExitStack

import numpy as np
import ml_dtypes

import concourse.bacc as bacc
import concourse.bass as bass
import concourse.mybir as mybir
import concourse.tile as tile
from concourse.bass_utils import run_bass_kernel_spmd
from concourse.masks import make_identity

FP = mybir.dt.float32
BF = mybir.dt.bfloat16
AF = mybir.ActivationFunctionType
ALU = mybir.AluOpType

B, S, D, NS, H = 2, 1024, 1024, 2, 16
DH = D // H  # 64
NC = 8
CH = D // NC  # 128 head-dims (2 heads) per core
GSZ = 4
RQ = S // GSZ  # 256 output rows per core
DA = 1152  # augmented contraction dim (1024 + bias row + zero pad), 9 * 128
KT = 8  # 128-tiles of S / D
KTA = 9  # 128-tiles of DA
SCALE = 1.0 / np.sqrt(D)  # score scale (reference uses sqrt(dim), not head_dim)
LN_EPS = 1e-5
GELU_FUNC = None  # set in _build_nc; overridable for CoreSim (no Gelu there)
BF_NP = ml_dtypes.bfloat16
BUILD_PHASES = "all"  # "proj" | "attn" | "a2a" | "all" — debugging aid
SKIP_CC = False  # replace the AllToAll with a local DMA copy (debugging aid)

_CACHE = {}


def _build_nc():
    nc = bacc.Bacc(
        "TRN2",
        target_bir_lowering=False,
        debug=False,
        enable_asserts=False,
        num_devices=NC,
    )

    # ---- I/O ----
    xta = [
        nc.dram_tensor(f"xta{b}", [DA, S], BF, kind="ExternalInput") for b in range(B)
    ]
    sta = [
        [
            nc.dram_tensor(f"sta{b}_{n}", [DA, S], BF, kind="ExternalInput")
            for n in range(NS)
        ]
        for b in range(B)
    ]
    wqa = nc.dram_tensor("wqa", [DA, CH], BF, kind="ExternalInput")
    wka = nc.dram_tensor("wka", [DA, CH], BF, kind="ExternalInput")
    wva = nc.dram_tensor("wva", [DA, CH], BF, kind="ExternalInput")
    sqa = [
        nc.dram_tensor(f"sq{n}a", [DA, CH], BF, kind="ExternalInput")
        for n in range(NS)
    ]
    ska = [
        nc.dram_tensor(f"sk{n}a", [DA, CH], BF, kind="ExternalInput")
        for n in range(NS)
    ]
    wo = nc.dram_tensor("wo", [D, D], BF, kind="ExternalInput")
    w1a = nc.dram_tensor("w1a", [DA, D], BF, kind="ExternalInput")
    w2a = nc.dram_tensor("w2a", [DA, D], BF, kind="ExternalInput")
    idr = nc.dram_tensor("idr", [RQ, D], FP, kind="ExternalInput")
    g1r = nc.dram_tensor("g1r", [1, D], BF, kind="ExternalInput")
    be1r = nc.dram_tensor("be1r", [1, D], BF, kind="ExternalInput")
    g2r = nc.dram_tensor("g2r", [1, D], BF, kind="ExternalInput")
    be2r = nc.dram_tensor("be2r", [1, D], BF, kind="ExternalInput")
    cmatd = nc.dram_tensor("cmatd", [3, 96], FP, kind="ExternalInput")
    out = nc.dram_tensor("out", [RQ, D], FP, kind="ExternalOutput")

    def t9(ap):  # [DA, N] -> [128, 9, N]
        return ap.rearrange("(kt p) n -> p kt n", p=128)

    def t8(ap):  # [D, N] -> [128, 8, N]
        return ap.rearrange("(kt p) n -> p kt n", p=128)

    with tile.TileContext(nc) as tc, ExitStack() as top:
        # ---------- persistent pools ----------
        const = top.enter_context(tc.tile_pool(name="const", bufs=1))
        glob = top.enter_context(tc.tile_pool(name="glob", bufs=1))

        ident = const.tile([128, 128], FP)
        make_identity(nc, ident)
        ones128 = const.tile([128, 128], BF)
        nc.gpsimd.memset(ones128[:], 1.0)
        ones_row = ones128[0:1, :]
        # selection matrices for Z row-sum packing: sel[:, n, m] = (m == n)
        sel = const.tile([128, 3, 3], BF)
        nc.gpsimd.memset(sel[:], 0.0)
        for n in range(3):
            nc.gpsimd.memset(sel[:, n, n : n + 1], 1.0)
        # C matrix (lhsT) for u1@row0, u2@row32, r0@row64 = exp(C.T @ lnZ)
        epsc = const.tile([128, 1], FP)
        nc.gpsimd.memset(epsc[:], LN_EPS)
        cmat = const.tile([3, 96], FP)
        nc.sync.dma_start(cmat[:], cmatd.ap())

        # tiles that span the collective boundary
        otl = [glob.tile([128, S], BF, name=f"otl{b}") for b in range(B)]
        otg = glob.tile([128, KT, RQ], BF)
        xln = glob.tile([128, 2, D], FP)

        # ---------- phase 1+2: projections + attention ----------
        with ExitStack() as p12:
            qkv = p12.enter_context(tc.tile_pool(name="qkv", bufs=1))
            qT = [qkv.tile([128, S], BF, name=f"qT{b}") for b in range(B)]
            kT = [qkv.tile([128, S], BF, name=f"kT{b}") for b in range(B)]
            sqT = [
                [qkv.tile([128, S], BF, name=f"sqT{b}_{n}") for n in range(NS)]
                for b in range(B)
            ]
            skT = [
                [qkv.tile([128, S], BF, name=f"skT{b}_{n}") for n in range(NS)]
                for b in range(B)
            ]
            vnat = [qkv.tile([128, KT, CH], BF, name=f"vnat{b}") for b in range(B)]

            with ExitStack() as p1:
                acts = p1.enter_context(tc.tile_pool(name="acts", bufs=2))
                wpool = p1.enter_context(tc.tile_pool(name="wpool", bufs=1))
                pps = p1.enter_context(tc.tile_pool(name="pps", bufs=2, space="PSUM"))
                vps = p1.enter_context(tc.tile_pool(name="vps", bufs=2, space="PSUM"))

                # weight tiles stay resident for both batches
                w_sb = {}
                for name, dram in (
                    ("q", wqa),
                    ("k", wka),
                    ("v", wva),
                    ("sq0", sqa[0]),
                    ("sk0", ska[0]),
                    ("sq1", sqa[1]),
                    ("sk1", ska[1]),
                ):
                    w = wpool.tile([128, KTA, CH], BF, name=f"w_{name}")
                    nc.sync.dma_start(w[:], t9(dram.ap()))
                    w_sb[name] = w

                def projT(src_sb, w, outT, on_act):
                    """outT[128, S] = W_aug.T @ src_aug."""
                    ps = pps.tile([128, S], FP, tag="pps", name="projps")
                    for kt in range(KTA):
                        for nn in range(2):
                            nc.tensor.matmul(
                                ps[:, nn * 512 : (nn + 1) * 512],
                                lhsT=w[:, kt, :],
                                rhs=src_sb[:, kt, nn * 512 : (nn + 1) * 512],
                                start=(kt == 0),
                                stop=(kt == KTA - 1),
                            )
                    if on_act:
                        nc.scalar.copy(outT[:], ps[:])
                    else:
                        nc.vector.tensor_copy(outT[:], ps[:])

                for b in range(B):
                    xta_sb = acts.tile([128, KTA, S], BF, tag="act", name="xta_sb")
                    # chunked: projections start after the first k-tiles land
                    nc.sync.dma_start(xta_sb[:, 0:3, :], t9(xta[b].ap())[:, 0:3, :])
                    nc.sync.dma_start(xta_sb[:, 3:6, :], t9(xta[b].ap())[:, 3:6, :])
                    nc.sync.dma_start(xta_sb[:, 6:KTA, :], t9(xta[b].ap())[:, 6:KTA, :])
                    projT(xta_sb, w_sb["q"], qT[b], True)
                    projT(xta_sb, w_sb["k"], kT[b], False)
                    for st in range(KT):
                        ps = vps.tile([128, CH], FP, tag="vps", name="vprojps")
                        for kt in range(KTA):
                            nc.tensor.matmul(
                                ps[:],
                                lhsT=xta_sb[:, kt, st * 128 : (st + 1) * 128],
                                rhs=w_sb["v"][:, kt, :],
                                start=(kt == 0),
                                stop=(kt == KTA - 1),
                            )
                        nc.vector.tensor_copy(vnat[b][:, st, :], ps[:])
                    for n in range(NS):
                        sta_sb = acts.tile([128, KTA, S], BF, tag="act", name="sta_sb")
                        nc.sync.dma_start(sta_sb[:], t9(sta[b][n].ap()))
                        projT(sta_sb, w_sb[f"sq{n}"], sqT[b][n], True)
                        projT(sta_sb, w_sb[f"sk{n}"], skT[b][n], False)

            # ----- attention: 2 batches x 2 heads -----
            with ExitStack() as p2:
                epool = p2.enter_context(tc.tile_pool(name="epool", bufs=2))
                spool = p2.enter_context(
                    tc.tile_pool(name="spool", bufs=2, space="PSUM")
                )
                mpool = p2.enter_context(
                    tc.tile_pool(name="mpool", bufs=2, space="PSUM")
                )
                small = p2.enter_context(tc.tile_pool(name="small", bufs=2))

                for b in range(B):
                    for h in range(2):
                        off = h * DH
                        qhs = [qT[b], sqT[b][0], sqT[b][1]]
                        khs = [kT[b], skT[b][0], skT[b][1]]
                        e = epool.tile([128, 3, KT, S], BF, tag="e", name="e")
                        zpk = mpool.tile([3, S], FP, tag="m", name="zpk")
                        for n in range(3):
                            qh = qhs[n][off : off + DH, :]  # [64, S]
                            kh = khs[n][off : off + DH, :]
                            for kt in range(KT):
                                sps = spool.tile([128, S], FP, tag="s", name="sps")
                                for nn in range(2):
                                    nc.tensor.matmul(
                                        sps[:, nn * 512 : (nn + 1) * 512],
                                        lhsT=kh[:, kt * 128 : (kt + 1) * 128],
                                        rhs=qh[:, nn * 512 : (nn + 1) * 512],
                                        start=True,
                                        stop=True,
                                    )
                                nc.scalar.activation(
                                    e[:, n, kt, :], sps[:], AF.Exp, scale=SCALE
                                )
                                for nn in range(2):
                                    nc.tensor.matmul(
                                        zpk[:, nn * 512 : (nn + 1) * 512],
                                        lhsT=sel[:, n, :],
                                        rhs=e[:, n, kt, nn * 512 : (nn + 1) * 512],
                                        start=(n == 0 and kt == 0),
                                        stop=(n == 2 and kt == KT - 1),
                                    )
                        lnz = small.tile([3, S], FP, tag="lnz", name="lnz")
                        nc.scalar.activation(lnz[:], zpk[:], AF.Ln)
                        wps = mpool.tile([96, S], FP, tag="m", name="wps")
                        for nn in range(2):
                            nc.tensor.matmul(
                                wps[:, nn * 512 : (nn + 1) * 512],
                                lhsT=cmat[:],
                                rhs=lnz[:, nn * 512 : (nn + 1) * 512],
                                start=True,
                                stop=True,
                            )
                        u = small.tile([96, S], BF, tag="u", name="u")
                        nc.scalar.activation(u[:], wps[:], AF.Exp)
                        # broadcast u1@0, u2@32 (bf16), r0@64 (fp32)
                        ub = []
                        for j, (rows, dt_) in enumerate(
                            ((128, BF), (128, BF), (64, BF))
                        ):
                            p0 = 32 * j
                            bps = mpool.tile([128, S], FP, tag="m", name=f"bps{j}")
                            for nn in range(2):
                                nc.tensor.matmul(
                                    bps[:rows, nn * 512 : (nn + 1) * 512],
                                    lhsT=ones128[p0 : p0 + 1, 0:rows],
                                    rhs=u[p0 : p0 + 1, nn * 512 : (nn + 1) * 512],
                                    start=True,
                                    stop=True,
                                )
                            ubj = small.tile([128, S], dt_, tag=f"ub{j}", name=f"ub{j}")
                            if j < 2:
                                nc.vector.tensor_copy(ubj[:], bps[:])
                            else:
                                nc.scalar.copy(ubj[:rows, :], bps[:rows, :])
                            ub.append(ubj)
                        # fuse: e0 = max(e0, u1*e1, u2*e2)
                        for kt in range(KT):
                            nc.vector.tensor_tensor(
                                e[:, 1, kt, :], e[:, 1, kt, :], ub[0][:], ALU.mult
                            )
                            nc.vector.tensor_tensor(
                                e[:, 2, kt, :], e[:, 2, kt, :], ub[1][:], ALU.mult
                            )
                            nc.vector.tensor_tensor(
                                e[:, 0, kt, :], e[:, 0, kt, :], e[:, 1, kt, :], ALU.max
                            )
                            nc.vector.tensor_tensor(
                                e[:, 0, kt, :], e[:, 0, kt, :], e[:, 2, kt, :], ALU.max
                            )
                        # O^T_h [64, S] accumulation over k tiles
                        ops = mpool.tile([64, S], FP, tag="m", name="ops")
                        for kt in range(KT):
                            for nn in range(2):
                                nc.tensor.matmul(
                                    ops[:, nn * 512 : (nn + 1) * 512],
                                    lhsT=vnat[b][:, kt, off : off + DH],
                                    rhs=e[:, 0, kt, nn * 512 : (nn + 1) * 512],
                                    start=(kt == 0),
                                    stop=(kt == KT - 1),
                                )
                        nc.vector.tensor_tensor(
                            otl[b][off : off + DH, :], ops[:], ub[2][0:DH, :], ALU.mult
                        )

        if BUILD_PHASES in ("proj", "attn"):
            out_t = out.ap().rearrange("(m p) d -> p m d", p=128)
            if BUILD_PHASES == "proj":
                for b in range(B):
                    nc.sync.dma_start(out_t[:, b, :], otl[b][:])  # garbage; just runs
            else:
                for b in range(B):
                    nc.sync.dma_start(out_t[:, b, :], otl[b][:])
            nc.compile()
            return nc

        # ---------- all-to-all: head-slices -> row-slices ----------
        with ExitStack() as p3:
            dpool = p3.enter_context(tc.tile_pool(name="dpool", bufs=1, space="DRAM"))
            ccin = dpool.tile([NC, CH, RQ], BF, name="ccin")
            ccout = dpool.tile([NC, CH, RQ], BF, name="ccout")
            for b in range(B):
                nc.sync.dma_start(
                    ccin[b * GSZ : (b + 1) * GSZ].rearrange("j p w -> p j w"),
                    otl[b].rearrange("p (j w) -> p j w", j=GSZ),
                )
            if SKIP_CC:
                nc.gpsimd.dma_start(ccout[:], ccin[:])
            else:
                nc.gpsimd.collective_compute(
                    "AllToAll",
                    ALU.bypass,
                    replica_groups=[list(range(NC))],
                    ins=[ccin.opt()],
                    outs=[ccout.opt()],
                )
            nc.sync.dma_start(otg[:], ccout.rearrange("i p w -> p i w"))

        if BUILD_PHASES == "a2a":
            out_t = out.ap().rearrange("(m p) d -> p m d", p=128)
            for m in range(2):
                nc.sync.dma_start(out_t[:, m, :], otg[:, 2*m:2*m+2, :].rearrange("p a w -> p (a w)"))
            nc.compile()
            return nc

        # ---------- phase 4: fc_o + residual + LN1 ----------
        def layer_norm(x_ps, res_sb, gb, bb, out_sb, pool):
            """out = LN(x_ps + res_sb) * gb + bb   (one 128-row tile)."""
            x = pool.tile([128, D], FP, tag="lnx", name="lnx")
            nc.vector.tensor_tensor(x[:], x_ps[:], res_sb[:], ALU.add)
            sm = pool.tile([128, 1], FP, tag="ln_sm", name="sm")
            nc.vector.reduce_sum(sm[:], x[:], axis=mybir.AxisListType.X)
            nm = pool.tile([128, 1], FP, tag="ln_nm", name="nm")
            nc.vector.tensor_scalar_mul(nm[:], sm[:], -1.0 / D)
            junk = pool.tile([128, D], FP, tag="lnj", name="junk")
            ssq = pool.tile([128, 1], FP, tag="ln_ssq", name="ssq")
            nc.vector.tensor_tensor(junk[:], x[:], x[:], ALU.mult)
            nc.vector.reduce_sum(ssq[:], junk[:], axis=mybir.AxisListType.X)
            msq = pool.tile([128, 1], FP, tag="ln_msq", name="msq")
            nc.vector.tensor_tensor(msq[:], nm[:], nm[:], ALU.mult)
            var = pool.tile([128, 1], FP, tag="ln_var", name="var")
            # var = ssq/D - mean^2  (nm = -mean, msq = mean^2)
            nc.vector.scalar_tensor_tensor(
                out=var[:],
                in0=ssq[:],
                scalar=1.0 / D,
                in1=msq[:],
                op0=ALU.mult,
                op1=ALU.subtract,
            )
            sqv = pool.tile([128, 1], FP, tag="ln_lnv", name="sqv")
            nc.scalar.activation(sqv[:], var[:], AF.Sqrt, bias=epsc[:])
            rstd = pool.tile([128, 1], FP, tag="ln_rstd", name="rstd")
            nc.vector.reciprocal(rstd[:], sqv[:])
            nmr = pool.tile([128, 1], FP, tag="ln_nmr", name="nmr")
            nc.vector.tensor_tensor(nmr[:], nm[:], rstd[:], ALU.mult)
            z = pool.tile([128, D], FP, tag="lnz2", name="z")
            nc.vector.tensor_scalar(z[:], x[:], nmr[:], rstd[:], ALU.add, ALU.mult)
            nc.vector.tensor_tensor(z[:], z[:], gb[:], ALU.mult)
            nc.vector.tensor_tensor(out_sb[:], z[:], bb[:], ALU.add)

        with ExitStack() as p4:
            wop = p4.enter_context(tc.tile_pool(name="wop", bufs=1))
            fps = p4.enter_context(tc.tile_pool(name="fps", bufs=2, space="PSUM"))
            lnp = p4.enter_context(tc.tile_pool(name="lnp", bufs=2))

            # affine broadcast tiles
            gb_tiles = []
            for nm_, dram in (("g1", g1r), ("be1", be1r), ("g2", g2r), ("be2", be2r)):
                row = lnp.tile([1, D], BF, tag="grow", name=f"{nm_}row")
                nc.sync.dma_start(row[:], dram.ap())
                bps = fps.tile([128, D], FP, tag="f", name=f"{nm_}bps")
                for nn in range(2):
                    nc.tensor.matmul(
                        bps[:, nn * 512 : (nn + 1) * 512],
                        lhsT=ones_row[:],
                        rhs=row[0:1, nn * 512 : (nn + 1) * 512],
                        start=True,
                        stop=True,
                    )
                gb = glob.tile([128, D], FP, name=f"{nm_}b")
                nc.scalar.copy(gb[:], bps[:])
                gb_tiles.append(gb)
            g1b, be1b, g2b, be2b = gb_tiles

            wo_sb = wop.tile([128, KT, D], BF)
            nc.sync.dma_start(wo_sb[:], t8(wo.ap()))
            idr_sb = glob.tile([128, 2, D], FP)
            nc.sync.dma_start(idr_sb[:], idr.ap().rearrange("(m p) d -> p m d", p=128))

            for m in range(2):
                ps = fps.tile([128, D], FP, tag="f", name="fcops")
                for kt in range(KT):
                    for nn in range(2):
                        nc.tensor.matmul(
                            ps[:, nn * 512 : (nn + 1) * 512],
                            lhsT=otg[:, kt, m * 128 : (m + 1) * 128],
                            rhs=wo_sb[:, kt, nn * 512 : (nn + 1) * 512],
                            start=(kt == 0),
                            stop=(kt == KT - 1),
                        )
                layer_norm(ps, idr_sb[:, m, :], g1b, be1b, xln[:, m, :], lnp)

        # ---------- phase 5: FF + residual + LN2 ----------
        with ExitStack() as p5:
            wfp = p5.enter_context(tc.tile_pool(name="wfp", bufs=1))
            hpool = p5.enter_context(tc.tile_pool(name="hpool", bufs=1))
            tps = p5.enter_context(tc.tile_pool(name="tps", bufs=2, space="PSUM"))
            f1ps = p5.enter_context(tc.tile_pool(name="f1ps", bufs=2, space="PSUM"))
            f2ps = p5.enter_context(tc.tile_pool(name="f2ps", bufs=2, space="PSUM"))
            lnp2 = p5.enter_context(tc.tile_pool(name="lnp2", bufs=2))

            # x_ln^T augmented [128, 9, RQ] (ones row at index 1024)
            xnt = hpool.tile([128, KTA, RQ], BF)
            nc.gpsimd.memset(xnt[:, KTA - 1, :], 0.0)
            nc.gpsimd.memset(xnt[0:1, KTA - 1, :], 1.0)
            for m in range(2):
                for dt_ in range(KT):
                    tp = tps.tile([128, 128], FP, tag="t", name="tp")
                    nc.tensor.transpose(
                        tp[:], xln[:, m, dt_ * 128 : (dt_ + 1) * 128], ident[:]
                    )
                    nc.vector.tensor_copy(xnt[:, dt_, m * 128 : (m + 1) * 128], tp[:])

            w1_sb = wfp.tile([128, KTA, D], BF, tag="w1")
            nc.sync.dma_start(w1_sb[:], t9(w1a.ap()))
            # h^T augmented [128, 9, RQ] with gelu applied
            hta = hpool.tile([128, KTA, RQ], BF)
            nc.gpsimd.memset(hta[:, KTA - 1, :], 0.0)
            nc.gpsimd.memset(hta[0:1, KTA - 1, :], 1.0)
            for mh in range(KT):
                ps = f1ps.tile([128, RQ], FP, tag="f1", name="ff1ps")
                for kt in range(KTA):
                    nc.tensor.matmul(
                        ps[:],
                        lhsT=w1_sb[:, kt, mh * 128 : (mh + 1) * 128],
                        rhs=xnt[:, kt, :],
                        start=(kt == 0),
                        stop=(kt == KTA - 1),
                    )
                nc.scalar.activation(hta[:, mh, :], ps[:], GELU_FUNC or AF.Gelu)

            w2_sb = wfp.tile([128, KTA, D], BF, tag="w2")
            nc.sync.dma_start(w2_sb[:], t9(w2a.ap()))
            out_t = out.ap().rearrange("(m p) d -> p m d", p=128)
            for m in range(2):
                ps = f2ps.tile([128, D], FP, tag="f2", name="ff2ps")
                for kt in range(KTA):
                    for nn in range(2):
                        nc.tensor.matmul(
                            ps[:, nn * 512 : (nn + 1) * 512],
                            lhsT=hta[:, kt, m * 128 : (m + 1) * 128],
                            rhs=w2_sb[:, kt, nn * 512 : (nn + 1) * 512],
                            start=(kt == 0),
                            stop=(kt == KTA - 1),
                        )
                osb = lnp2.tile([128, D], FP, tag="osb", name="osb")
                layer_norm(ps, xln[:, m, :], g2b, be2b, osb, lnp2)
                nc.sync.dma_start(out_t[:, m, :], osb[:])

    nc.compile()
    return nc


def _cmat_np():
    c = np.zeros((3, 96), np.float32)
    c[0, 0], c[1, 0] = 1.0, -1.0   # u1 = Z0/Z1 @ row 0
    c[0, 32], c[2, 32] = 1.0, -1.0  # u2 = Z0/Z2 @ row 32
    c[0, 64] = -1.0                 # r0 = 1/Z0 @ row 64
    return c


def _prep_inputs(id_x, side, Wq, bq, Wk, bk, Wv, bv, sWq, sbq, sWk, sbk, Wo, bo,
                 W1, b1, W2, b2, g1, be1, g2, be2):
    """Build the 8 per-core input maps."""
    f = np.float32
    id_x = np.asarray(id_x, f)
    side = np.asarray(side, f)

    def aug_act(xt):  # [D, S] -> [DA, S] bf16 with ones row at D
        a = np.zeros((DA, S), f)
        a[:D] = xt
        a[D] = 1.0
        return a.astype(BF_NP)

    def aug_w(W, b_, cs):  # [D, D], [D] -> [DA, CH] bf16
        a = np.zeros((DA, CH), f)
        a[:D] = np.asarray(W, f)[:, cs]
        a[D] = np.asarray(b_, f)[cs]
        return a.astype(BF_NP)

    def aug_w_full(W, b_):  # [D, D], [D] -> [DA, D] bf16
        a = np.zeros((DA, D), f)
        a[:D] = np.asarray(W, f)
        a[D] = np.asarray(b_, f)
        return a.astype(BF_NP)

    xta_b = [aug_act(np.ascontiguousarray(id_x[b].T)) for b in range(B)]
    sta_b = [
        [aug_act(np.ascontiguousarray(side[b, :, n, :].T)) for n in range(NS)]
        for b in range(B)
    ]
    w1a = aug_w_full(W1, b1)
    w2a = aug_w_full(W2, b2)
    wo_f = np.ascontiguousarray(np.asarray(Wo, f)).astype(BF_NP)
    bo_f = np.asarray(bo, f)

    in_maps = []
    for cid in range(NC):
        bq_, g = cid // GSZ, cid % GSZ
        cs = slice(cid * CH, (cid + 1) * CH)
        rows = slice(g * RQ, (g + 1) * RQ)
        m = {
            "xta0": xta_b[0],
            "xta1": xta_b[1],
            "sta0_0": sta_b[0][0],
            "sta0_1": sta_b[0][1],
            "sta1_0": sta_b[1][0],
            "sta1_1": sta_b[1][1],
            "wqa": aug_w(Wq, bq, cs),
            "wka": aug_w(Wk, bk, cs),
            "wva": aug_w(Wv, bv, cs),
            "sq0a": aug_w(np.asarray(sWq, f)[0], np.asarray(sbq, f)[0], cs),
            "sk0a": aug_w(np.asarray(sWk, f)[0], np.asarray(sbk, f)[0], cs),
            "sq1a": aug_w(np.asarray(sWq, f)[1], np.asarray(sbq, f)[1], cs),
            "sk1a": aug_w(np.asarray(sWk, f)[1], np.asarray(sbk, f)[1], cs),
            "wo": wo_f,
            "w1a": w1a,
            "w2a": w2a,
            "idr": np.ascontiguousarray(id_x[bq_, rows, :]) + bo_f[None, :].astype(f),
            "cmatd": _cmat_np(),
            "g1r": np.asarray(g1, f).reshape(1, D).astype(BF_NP),
            "be1r": np.asarray(be1, f).reshape(1, D).astype(BF_NP),
            "g2r": np.asarray(g2, f).reshape(1, D).astype(BF_NP),
            "be2r": np.asarray(be2, f).reshape(1, D).astype(BF_NP),
        }
        in_maps.append(m)
    return in_maps


def kernel(id_x, side, num_heads, Wq, bq, Wk, bk, Wv, bv, sWq, sbq, sWk, sbk,
           Wo, bo, W1, b1, W2, b2, g1, be1, g2, be2, _trace=False):
    assert int(num_heads) == H
    in_maps = _prep_inputs(id_x, side, Wq, bq, Wk, bk, Wv, bv, sWq, sbq, sWk, sbk,
                           Wo, bo, W1, b1, W2, b2, g1, be1, g2, be2)
    if "nc" not in _CACHE:
        _CACHE["nc"] = _build_nc()
    nc = _CACHE["nc"]
    res = run_bass_kernel_spmd(nc, in_maps, core_ids=list(range(NC)), trace=_trace)
    _CACHE["last_result"] = res
    outp = np.zeros((B, S, D), np.float32)
    for cid in range(NC):
        b, g = cid // GSZ, cid % GSZ
        outp[b, g * RQ : (g + 1) * RQ, :] = res.results[cid]["out"]
    return outp

